# revision 29
# baseline (speedup 1.0000x reference)
"""Trainium2 Bass kernel for nn_DeformConv (DCNv2 3x3 + BN(eval) + ReLU).

Problem (hardcoded): x [4, 256, 64, 64] f32; offset conv w_off [27, 256, 3, 3];
main conv w [256, 256, 3, 3]; BN params [256]. Output [4, 256, 64, 64] f32.

Sharding: 8 cores; core c handles sample b = c//2, output rows
h0 = 32*(c%2) .. h0+32 (2048 output pixels per core). Params replicated.

Per-core algorithm (all on device, one Bass program, SPMD over 8 cores):
  1. offset conv om[27, 2048] = sum over 9 taps of WoffT.T @ shifted padded x
     (PE, PSUM-accumulated, N=512 groups)
  2. PE-transpose om -> omT [2048 pix, 27] (pixel-major)
  3. coords on DVE/ACT: py/px = base + off; ly = mod(py,1); y0 = py - ly;
     mask = sigmoid; bilinear weights w00..w11 = f(ly, lx)*mask;
     gather row index = (y0+PAD)*Wp + x0+PAD (int32)
  4. indirect-DMA gather from a pixel-major zero-padded DRAM table
     [Hp*Wp, C]: per index, 2C contiguous floats = both x-corners of one
     y-row; two indices (top/bottom row) per (pixel, tap)
  5. "scaled transpose" on PE: S_k[cchunk, n] += gathered_chunk.T @ diag(w_i)
     accumulating all 4 corners in PSUM -> channel-major sampled tensor
     (bilinear combine and transpose fused into matmuls)
  6. main conv: out[O, n] = sum_{k, cchunk} WmatT.T @ S (BN scale folded into
     weights on host), then ACT applies Relu(out + shift) during PSUM->SBUF
  7. DMA out
"""
import functools
import numpy as np

import concourse.bass as bass
import concourse.bacc as bacc
import concourse.tile as tile
import concourse.mybir as mybir
from concourse.masks import make_identity

# ---------------- problem constants (hardcoded per contract) ----------------
B, C, H, W = 4, 256, 64, 64
O = 256
KK = 9
BN_EPS = 1e-5
NCORES = 8
ROWS = 32                 # output rows per core
N = ROWS * W              # 2048 output pixels per core
PAD = 8                   # table padding (max |offset| measured ~2.35)
Hp, Wp = H + 2 * PAD, W + 2 * PAD
VROWS = Hp * Wp + Wp + 2  # table rows (+ slack so idx+Wp reads stay in range)
NCH = 16                  # pixel chunks of 128 per core
G4 = 4                    # chunk groups of 4 (512 output pixels)

F32 = mybir.dt.float32
F32R = mybir.dt.float32r
F16 = mybir.dt.float16
I32 = mybir.dt.int32
I16 = mybir.dt.int16

# ---------------- dtype knobs ----------------
TABLE_DT = F16   # dtype of gather table in DRAM (and gathered tiles)
DIAG_DT = F16    # dtype of diag weight matrices (must match gathered for MM)
MAIN_DT = F16    # dtype of S staging + main conv weights
OFF_F16 = True   # run offset conv in fp16 (device-side cast)


def _np_dt(dt):
    return {F32: np.float32, F16: np.float16}[dt]


def build_nc(floor_bias=-0.5):
    nc = bacc.Bacc("TRN2", target_bir_lowering=False, debug=False,
                   num_devices=NCORES, num_swdge_queues=4)

    # ---- per-core DRAM parameters ----
    table = nc.dram_tensor("table", [VROWS, 4 * C], TABLE_DT, kind="ExternalInput")
    # f16 blob: [xpad 2*2244 | wofft 2*243] (pre-converted on host)
    XW_LEN = 2 * 2244 + 2 * 243
    xw16 = nc.dram_tensor("xw16", [128, XW_LEN], F16, kind="ExternalInput")
    # one f32 const blob per partition: [basey 144 | basex 144 | shift 2 | boff 1]
    CB_BY = 0
    CB_BX = CB_BY + NCH * KK
    CB_SH = CB_BX + NCH * KK
    CB_BO = CB_SH + 2
    CB_LEN = CB_BO + 1
    cblob = nc.dram_tensor("cblob", [128, CB_LEN], F32, kind="ExternalInput")
    wmat = nc.dram_tensor("wmat", [2, 128, KK, O], MAIN_DT, kind="ExternalInput")
    yout = nc.dram_tensor("yout", [2, 128, N], F32, kind="ExternalOutput")

    AF = mybir.ActivationFunctionType
    ALU = mybir.AluOpType

    with tile.TileContext(nc) as tc:
        with (
            tc.tile_pool(name="const", bufs=1) as const,
            tc.tile_pool(name="coord", bufs=1) as coord,
            tc.tile_pool(name="gat", bufs=6) as gat,
            tc.tile_pool(name="diagp", bufs=2) as diagp,
            tc.tile_pool(name="ssb", bufs=2) as ssb,
            tc.tile_pool(name="ysb", bufs=2) as ysb,
            tc.tile_pool(name="wvp", bufs=2) as wvp,
            tc.tile_pool(name="ps_misc", bufs=2, space="PSUM") as ps_misc,
            tc.tile_pool(name="ps_s", bufs=1, space="PSUM") as ps_s,
            tc.tile_pool(name="ps_y", bufs=1, space="PSUM") as ps_y,
            tc.tile_pool(name="ps_T", bufs=2, space="PSUM") as ps_T,
        ):
            # ---------------- load constants ----------------
            xw = const.tile([128, XW_LEN], F16)
            nc.sync.dma_start(out=xw[:], in_=xw16[:])
            cb = const.tile([128, CB_LEN], F32)
            nc.sync.dma_start(out=cb[:], in_=cblob[:])
            basey_t = cb[:, CB_BY:CB_BX]
            basex_t = cb[:, CB_BX:CB_SH]
            shift_t = cb[:, CB_SH:CB_BO]
            boff_t = cb[:27, CB_BO:CB_BO + 1]
            wmat_t = const.tile([128, 2, KK * O], MAIN_DT)
            nc.sync.dma_start(
                out=wmat_t[:], in_=wmat[:].rearrange("a p k o -> p a (k o)"))

            ident = const.tile([128, 128], F32)
            make_identity(nc, ident[:])
            if DIAG_DT != F32:
                identd = const.tile([128, 128], DIAG_DT)
                nc.vector.tensor_copy(identd[:], ident[:])
            else:
                identd = ident

            # ---------------- per-group pipeline ----------------
            # For each 512-pixel group g4: offset-conv group -> omT ->
            # coords/weights/indices -> gathers + scaled-T + main conv.
            # Group g4+1's prologue overlaps group g4's gathers/compute.
            xv = xw[:, 0:2 * 2244].rearrange("p (a r w) -> p a r w",
                                             a=2, r=34, w=66)
            wof = xw[:, 2 * 2244:].rearrange("p (a f) -> p a f", a=2)
            CNAMES = ("00", "01", "10", "11")
            FD = 4 * KK  # 36 per group
            # idx16r[q, g4*288 + kk*32 + c*8 + r] (int16, stripe-replicated)
            idx16r = coord.tile([128, G4 * KK * 32], I16)

            for g4 in range(G4):
                # --- offset conv for this group (8 output rows) ---
                ps = ps_misc.tile([27, 512], F32, name="psom", tag="psmisc")
                first = True
                for kk in range(KK):
                    ki, kj = kk // 3, kk % 3
                    for cc in range(2):
                        rhs = xv[:, cc, g4 * 8 + ki:g4 * 8 + ki + 8,
                                 kj:kj + 64]
                        lhsT = wof[:, cc, kk * 27:(kk + 1) * 27]
                        nc.tensor.matmul(
                            ps[:], lhsT=lhsT, rhs=rhs,
                            start=first, stop=(kk == KK - 1 and cc == 1))
                        first = False
                om_g = coord.tile([27, 512], F32, name="om_g", tag="om_g")
                nc.scalar.activation(om_g[:], ps[:],
                                     AF.Identity, bias=boff_t, scale=1.0)

                # --- omT for the 4 chunks of this group ---
                omT = coord.tile([128, 4, 27], F32, name="omT", tag="omT")
                for c in range(4):
                    pst = ps_misc.tile([128, 27], F32, name="pst",
                                       tag="psmisc")
                    nc.tensor.transpose(pst[:], om_g[:, c * 128:(c + 1) * 128],
                                        ident[:27, :27])
                    nc.vector.tensor_copy(omT[:, c, :], pst[:])

                # --- coords / weights / indices ([128, 36] tiles) ---
                _ntc = [0]

                def nt(shape=(128, FD), dt=F32):
                    _ntc[0] += 1
                    return coord.tile(list(shape), dt, name=f"ct{_ntc[0]}",
                                      tag=f"ct{_ntc[0]}")

                bsl = slice(g4 * FD, (g4 + 1) * FD)
                py = nt()
                px = nt()
                nc.vector.tensor_tensor(py[:], omT[:, :, 0:9],
                                        basey_t[:, bsl], op=ALU.add)
                nc.vector.tensor_tensor(px[:], omT[:, :, 9:18],
                                        basex_t[:, bsl], op=ALU.add)
                msk = nt()
                nc.scalar.activation(msk[:], omT[:, :, 18:27], AF.Sigmoid)
                # floor: HW f32->i32 convert rounds-to-nearest, so convert
                # (py - 0.5): round(py - 0.5) == floor(py) (coords > 0;
                # py - 0.5 is exact in fp32 at this magnitude)
                y0i = nt((128, FD), I32)
                x0i = nt((128, FD), I32)
                nc.vector.tensor_scalar(y0i[:], py[:], floor_bias, None,
                                        op0=ALU.add)
                nc.vector.tensor_scalar(x0i[:], px[:], floor_bias, None,
                                        op0=ALU.add)
                y0 = nt(); x0 = nt()
                nc.vector.tensor_copy(y0[:], y0i[:])
                nc.vector.tensor_copy(x0[:], x0i[:])
                ly = nt(); lx = nt()
                nc.vector.tensor_tensor(ly[:], py[:], y0[:], op=ALU.subtract)
                nc.vector.tensor_tensor(lx[:], px[:], x0[:], op=ALU.subtract)
                # weights: wtop = m*(1-ly), wbot = m*ly; w00 = wtop*(1-lx)...
                wbot = nt(); wtop = nt()
                nc.vector.tensor_tensor(wbot[:], ly[:], msk[:], op=ALU.mult)
                nc.vector.tensor_tensor(wtop[:], msk[:], wbot[:],
                                        op=ALU.subtract)
                t0 = nt(); t1 = nt()
                nc.vector.tensor_tensor(t0[:], wtop[:], lx[:], op=ALU.mult)
                nc.vector.tensor_tensor(t1[:], wbot[:], lx[:], op=ALU.mult)
                tw = {}
                for nm in CNAMES:
                    tw[nm] = nt((128, FD), F32)
                nc.vector.tensor_copy(tw["01"][:], t0[:])
                nc.vector.tensor_copy(tw["11"][:], t1[:])
                nc.vector.tensor_tensor(tw["00"][:], wtop[:], t0[:],
                                        op=ALU.subtract)
                nc.vector.tensor_tensor(tw["10"][:], wbot[:], t1[:],
                                        op=ALU.subtract)
                # f16 corner weights, k-major [128, KK, 4]
                wv16 = {}
                for nm in CNAMES:
                    wt = wvp.tile([128, KK, 4], F16, name=f"wv16{nm}",
                                  tag=f"wv16{nm}")
                    nc.vector.tensor_copy(
                        wt[:], tw[nm][:].rearrange("p (c k) -> p k c", k=KK))
                    wv16[nm] = wt
                # gather indices: idx = y0*Wp + x0 (+PAD offsets in base)
                idxf = nt()
                nc.vector.tensor_scalar(idxf[:], y0[:], float(Wp), None,
                                        op0=ALU.mult)
                nc.vector.tensor_tensor(idxf[:], idxf[:], x0[:], op=ALU.add)

                # 16-wrap the indices: two-stage PE transpose
                idxv = idxf[:].rearrange("p (c k) -> p k c", k=KK)
                for kk in range(KK):
                    psa = ps_T.tile([4, 128], F32, name="psT1", tag="psT")
                    nc.tensor.transpose(psa[:], idxv[:, kk, :], ident[:])
                    a_sb = coord.tile([4, 128], F32, name="aT1", tag="aT1")
                    nc.vector.tensor_copy(a_sb[:], psa[:])
                    pst2 = ps_T.tile([16, 32], F32, name="psT2", tag="psT")
                    for r in range(8):
                        nc.tensor.transpose(pst2[:, r * 4:(r + 1) * 4],
                                            a_sb[:, r * 16:(r + 1) * 16],
                                            ident[:4, :4])
                    base = g4 * KK * 32 + kk * 32
                    nc.vector.tensor_copy(
                        idx16r[0:16, base:base + 32]
                        .rearrange("q (c r) -> q c r", r=8),
                        pst2[:].rearrange("q (r c) -> q c r", r=8))
                # replicate this group's indices to all 8 gpsimd stripes
                gb = slice(g4 * KK * 32, (g4 + 1) * KK * 32)
                for st in (16, 32, 64):
                    nc.sync.dma_start(out=idx16r[st:2 * st, gb],
                                      in_=idx16r[0:st, gb])

                # --- gathers + scaled transposes + main conv ---
                psy = [ps_y.tile([128, 512], F32, name=f"psy{oc_}",
                                 tag=f"psy{oc_}") for oc_ in range(2)]
                for kk in range(KK):
                    s_sb = ssb.tile([128, 2, 512], MAIN_DT)
                    # gather 512 2x2-patch rows (4C f16 = 2KB each) in one
                    # dma_gather; output layout matches chunk/pixel-major
                    gt = gat.tile([128, 4, 4 * C], TABLE_DT)
                    nc.gpsimd.dma_gather(
                        out_ap=gt[:],
                        in_ap=table[:],
                        idxs_ap=idx16r[:, g4 * KK * 32 + kk * 32:
                                       g4 * KK * 32 + (kk + 1) * 32],
                        num_idxs=512, num_idxs_reg=512, elem_size=4 * C,
                        queue_num=(g4 * KK + kk) % 4)
                    # diag weight matrices (4 chunks per op, broadcast APs)
                    # + scaled transposes
                    ps_cc = [ps_s.tile([128, 512], F32, name=f"sps{cc_}",
                                       tag=f"sps{cc_}") for cc_ in range(2)]
                    dg4 = {}
                    for j, nm in enumerate(CNAMES):
                        d4 = diagp.tile([128, 4, 128], DIAG_DT,
                                        tag=f"diag{nm}")
                        nc.vector.tensor_tensor(
                            d4[:],
                            identd[:].rearrange("p (a f) -> p a f", a=1)
                            .to_broadcast([128, 4, 128]),
                            wv16[nm][:, kk, :]
                            .rearrange("p (c o) -> p c o", o=1)
                            .to_broadcast([128, 4, 128]),
                            op=ALU.mult)
                        dg4[(j // 2, j % 2)] = d4
                    for c4 in range(4):
                        for tb in range(2):
                            for xh in range(2):
                                for cc in range(2):
                                    base = (tb * 2 + xh) * 256 + cc * 128
                                    nc.tensor.matmul(
                                        ps_cc[cc][:, c4 * 128:(c4 + 1) * 128],
                                        lhsT=gt[:, c4, base:base + 128],
                                        rhs=dg4[(tb, xh)][:, c4, :],
                                        start=(c4 == 0 and tb == 0 and xh == 0),
                                        stop=(c4 == 3 and tb == 1 and xh == 1),
                                    )
                    for cc in range(2):
                        nc.scalar.activation(s_sb[:, cc, :], ps_cc[cc][:],
                                             AF.Copy)
                    # main conv contribution of this tap (PSUM-accumulated)
                    for oc in range(2):
                        for cc in range(2):
                            nc.tensor.matmul(
                                psy[oc][:],
                                lhsT=wmat_t[:, cc, kk * O + oc * 128:
                                            kk * O + (oc + 1) * 128],
                                rhs=s_sb[:, cc, :],
                                start=(kk == 0 and cc == 0),
                                stop=(kk == KK - 1 and cc == 1))

                y_sb = ysb.tile([128, 2, 512], F32)
                for oc in range(2):
                    nc.scalar.activation(y_sb[:, oc, :], psy[oc][:], AF.Relu,
                                         bias=shift_t[:, oc:oc + 1], scale=1.0)
                    nc.sync.dma_start(
                        out=yout[oc][:, g4 * 512:(g4 + 1) * 512],
                        in_=y_sb[:, oc, :])
    nc.compile()
    return nc


@functools.lru_cache(maxsize=1)
def _cached_nc():
    return build_nc()


def prep_core_inputs(inputs):
    """Host-side prep: per-core input maps (numpy only)."""
    x = np.asarray(inputs["x"], np.float32)
    w_off = np.asarray(inputs["w_off"], np.float32)
    b_off = np.asarray(inputs["b_off"], np.float32)
    w = np.asarray(inputs["w"], np.float32)
    b = np.asarray(inputs["b"], np.float32)
    gamma = np.asarray(inputs["gamma"], np.float32)
    beta = np.asarray(inputs["beta"], np.float32)
    rm = np.asarray(inputs["running_mean"], np.float32)
    rv = np.asarray(inputs["running_var"], np.float32)

    tdt = _np_dt(TABLE_DT)
    mdt = _np_dt(MAIN_DT)

    tables = []
    for bb in range(B):
        # flat padded pixel-major image with slack rows
        flat = np.zeros((VROWS + Wp + 2, C), np.float32)
        img = np.zeros((Hp, Wp, C), np.float32)
        img[PAD:PAD + H, PAD:PAD + W, :] = x[bb].transpose(1, 2, 0)
        flat[:Hp * Wp] = img.reshape(Hp * Wp, C)
        # patch table: row r = [r, r+1, r+Wp, r+Wp+1]
        t = np.empty((VROWS, 4 * C), tdt)
        t[:, 0 * C:1 * C] = flat[0:VROWS]
        t[:, 1 * C:2 * C] = flat[1:VROWS + 1]
        t[:, 2 * C:3 * C] = flat[Wp:VROWS + Wp]
        t[:, 3 * C:4 * C] = flat[Wp + 1:VROWS + Wp + 1]
        tables.append(np.ascontiguousarray(t))

    # offset conv weights: wofft[cc, p, kk, :] = w_off[:, cc*128+p, ki, kj]
    wofft = np.ascontiguousarray(
        w_off.reshape(27, 2, 128, 3, 3).transpose(1, 2, 3, 4, 0)
        .reshape(2, 128, 3 * 3 * 27)).astype(np.float32)

    # main conv weights, BN-folded: wmat[cc, p, kk, o] = w[o, cc*128+p, kk]*inv[o]
    inv = gamma / np.sqrt(rv + BN_EPS)
    shift = b * inv + beta - rm * inv
    wk = (w.reshape(O, C, KK) * inv[:, None, None]).astype(np.float32)
    wmat = np.ascontiguousarray(
        wk.reshape(O, 2, 128, KK).transpose(1, 2, 3, 0)).astype(mdt)

    shiftp = np.ascontiguousarray(shift.reshape(2, 128).T).astype(np.float32)
    boffp = np.zeros((128, 1), np.float32)
    boffp[:27, 0] = b_off

    in_maps = []
    for core in range(NCORES):
        bb = core // 2
        h0 = ROWS * (core % 2)
        # base grids [128, NCH*KK]: pixel n = c*128+p; h = h0 + n//64; w = n%64
        p = np.arange(128)
        cgrid = np.arange(NCH)
        kk = np.arange(KK)
        hh = (h0 + 2 * cgrid[None, :, None] + p[:, None, None] // 64)
        wwc = (p[:, None, None] % 64) * np.ones((1, NCH, 1))
        ki = (kk // 3)[None, None, :]
        kj = (kk % 3)[None, None, :]
        by = (hh - 1.0 + ki + PAD).astype(np.float32).reshape(128, NCH * KK)
        bx = (wwc - 1.0 + kj + PAD).astype(np.float32).reshape(128, NCH * KK)

        # xpad window rows h0-1 .. h0+32, zero-padded, 66 cols
        xp = np.zeros((2, 128, 34, 66), np.float32)
        r0 = max(0, h0 - 1); r1 = min(H, h0 + 33)
        xp[:, :, (r0 - (h0 - 1)):(r1 - (h0 - 1)), 1:W + 1] = (
            x[bb].reshape(2, 128, H, W)[:, :, r0:r1, :])

        # f16 blob [xpad 2*2244 | wofft 2*243]; f32 blob [by|bx|shift|boff]
        xw16 = np.concatenate([
            xp.reshape(2, 128, 34 * 66).transpose(1, 0, 2).reshape(128, -1),
            wofft.transpose(1, 0, 2).reshape(128, -1),
        ], axis=1).astype(np.float16)
        cblob = np.concatenate([by, bx, shiftp, boffp],
                               axis=1).astype(np.float32)

        in_maps.append(dict(
            table=tables[bb],
            xw16=np.ascontiguousarray(xw16),
            cblob=np.ascontiguousarray(cblob),
            wmat=wmat,
        ))
    return in_maps


def assemble_output(results):
    y = np.zeros((B, O, H, W), np.float32)
    for core in range(NCORES):
        bb = core // 2
        h0 = ROWS * (core % 2)
        yo = results[core]["yout"]  # [2, 128, N]
        y[bb, :, h0:h0 + ROWS, :] = yo.reshape(O, ROWS, W)
    return y


def kernel(**inputs):
    from concourse.bass_utils import run_bass_kernel_spmd
    nc = _cached_nc()
    in_maps = prep_core_inputs(inputs)
    res = run_bass_kernel_spmd(nc, in_maps, core_ids=list(range(NCORES)))
    return assemble_output(res.results)



# revision 32
# speedup vs baseline: 1.1034x; 1.1034x over previous
"""Trainium2 Bass kernel for nn_DeformConv (DCNv2 3x3 + BN(eval) + ReLU).

Problem (hardcoded): x [4, 256, 64, 64] f32; offset conv w_off [27, 256, 3, 3];
main conv w [256, 256, 3, 3]; BN params [256]. Output [4, 256, 64, 64] f32.

Sharding: 8 cores; core c handles sample b = c//2, output rows
h0 = 32*(c%2) .. h0+32 (2048 output pixels per core). Params replicated.

Per-core algorithm (all on device, one Bass program, SPMD over 8 cores):
  1. offset conv om[27, 2048] = sum over 9 taps of WoffT.T @ shifted padded x
     (PE, PSUM-accumulated, N=512 groups)
  2. PE-transpose om -> omT [2048 pix, 27] (pixel-major)
  3. coords on DVE/ACT: py/px = base + off; ly = mod(py,1); y0 = py - ly;
     mask = sigmoid; bilinear weights w00..w11 = f(ly, lx)*mask;
     gather row index = (y0+PAD)*Wp + x0+PAD (int32)
  4. indirect-DMA gather from a pixel-major zero-padded DRAM table
     [Hp*Wp, C]: per index, 2C contiguous floats = both x-corners of one
     y-row; two indices (top/bottom row) per (pixel, tap)
  5. "scaled transpose" on PE: S_k[cchunk, n] += gathered_chunk.T @ diag(w_i)
     accumulating all 4 corners in PSUM -> channel-major sampled tensor
     (bilinear combine and transpose fused into matmuls)
  6. main conv: out[O, n] = sum_{k, cchunk} WmatT.T @ S (BN scale folded into
     weights on host), then ACT applies Relu(out + shift) during PSUM->SBUF
  7. DMA out
"""
import functools
import numpy as np

import concourse.bass as bass
import concourse.bacc as bacc
import concourse.tile as tile
import concourse.mybir as mybir
from concourse.masks import make_identity

# ---------------- problem constants (hardcoded per contract) ----------------
B, C, H, W = 4, 256, 64, 64
O = 256
KK = 9
BN_EPS = 1e-5
NCORES = 8
ROWS = 32                 # output rows per core
N = ROWS * W              # 2048 output pixels per core
PAD = 8                   # table padding (max |offset| measured ~2.35)
Hp, Wp = H + 2 * PAD, W + 2 * PAD
VROWS = Hp * Wp + Wp + 2  # table rows (+ slack so idx+Wp reads stay in range)
NCH = 16                  # pixel chunks of 128 per core
G4 = 4                    # chunk groups of 4 (512 output pixels)

F32 = mybir.dt.float32
F32R = mybir.dt.float32r
F16 = mybir.dt.float16
I32 = mybir.dt.int32
I16 = mybir.dt.int16

# ---------------- dtype knobs ----------------
TABLE_DT = F16   # dtype of gather table in DRAM (and gathered tiles)
DIAG_DT = F16    # dtype of diag weight matrices (must match gathered for MM)
MAIN_DT = F16    # dtype of S staging + main conv weights
OFF_F16 = True   # run offset conv in fp16 (device-side cast)


def _np_dt(dt):
    return {F32: np.float32, F16: np.float16}[dt]


def build_nc(floor_bias=-0.5):
    nc = bacc.Bacc("TRN2", target_bir_lowering=False, debug=False,
                   num_devices=NCORES, num_swdge_queues=4)

    # ---- per-core DRAM parameters ----
    table = nc.dram_tensor("table", [VROWS, 4 * C], TABLE_DT, kind="ExternalInput")
    # f16 blob: [xpad 2*2244 | wofft 2*243] (pre-converted on host)
    XW_LEN = 2 * 2244 + 2 * 243
    xw16 = nc.dram_tensor("xw16", [128, XW_LEN], F16, kind="ExternalInput")
    # one f32 const blob per partition: [basey 144 | basex 144 | shift 2 | boff 1]
    CB_BY = 0
    CB_BX = CB_BY + NCH * KK
    CB_SH = CB_BX + NCH * KK
    CB_BO = CB_SH + 2
    CB_LEN = CB_BO + 1
    cblob = nc.dram_tensor("cblob", [128, CB_LEN], F32, kind="ExternalInput")
    wmat = nc.dram_tensor("wmat", [2, 128, KK, O], MAIN_DT, kind="ExternalInput")
    yout = nc.dram_tensor("yout", [2, 128, N], F32, kind="ExternalOutput")

    AF = mybir.ActivationFunctionType
    ALU = mybir.AluOpType

    with tile.TileContext(nc) as tc:
        with (
            tc.tile_pool(name="const", bufs=1) as const,
            tc.tile_pool(name="coord", bufs=1) as coord,
            tc.tile_pool(name="gat", bufs=4) as gat,
            tc.tile_pool(name="diagp", bufs=2) as diagp,
            tc.tile_pool(name="ssb", bufs=2) as ssb,
            tc.tile_pool(name="ysb", bufs=2) as ysb,
            tc.tile_pool(name="wvp", bufs=2) as wvp,
            tc.tile_pool(name="idxp", bufs=2) as idxp,
            tc.tile_pool(name="ps_misc", bufs=2, space="PSUM") as ps_misc,
            tc.tile_pool(name="ps_s", bufs=1, space="PSUM") as ps_s,
            tc.tile_pool(name="ps_y", bufs=1, space="PSUM") as ps_y,
            tc.tile_pool(name="ps_T", bufs=2, space="PSUM") as ps_T,
        ):
            # ---------------- load constants ----------------
            xw = const.tile([128, XW_LEN], F16)
            nc.sync.dma_start(out=xw[:], in_=xw16[:])
            cb = const.tile([128, CB_LEN], F32)
            nc.sync.dma_start(out=cb[:], in_=cblob[:])
            basey_t = cb[:, CB_BY:CB_BX]
            basex_t = cb[:, CB_BX:CB_SH]
            shift_t = cb[:, CB_SH:CB_BO]
            boff_t = cb[:27, CB_BO:CB_BO + 1]
            wmat_t = const.tile([128, 2, KK * O], MAIN_DT)
            nc.sync.dma_start(
                out=wmat_t[:], in_=wmat[:].rearrange("a p k o -> p a (k o)"))

            ident = const.tile([128, 128], F32)
            make_identity(nc, ident[:])
            if DIAG_DT != F32:
                identd = const.tile([128, 128], DIAG_DT)
                nc.vector.tensor_copy(identd[:], ident[:])
            else:
                identd = ident

            # ---------------- per-group pipeline ----------------
            # For each 512-pixel group g4: offset-conv group -> omT ->
            # coords/weights/indices -> gathers + scaled-T + main conv.
            # Group g4+1's prologue overlaps group g4's gathers/compute.
            xv = xw[:, 0:2 * 2244].rearrange("p (a r w) -> p a r w",
                                             a=2, r=34, w=66)
            wof = xw[:, 2 * 2244:].rearrange("p (a f) -> p a f", a=2)
            CNAMES = ("00", "01", "10", "11")
            FD = 4 * KK  # 36 per group

            def prep_group(g4):
                # --- offset conv for this group (8 output rows) ---
                ps = ps_misc.tile([27, 512], F32, name="psom", tag="psmisc")
                first = True
                for kk in range(KK):
                    ki, kj = kk // 3, kk % 3
                    for cc in range(2):
                        rhs = xv[:, cc, g4 * 8 + ki:g4 * 8 + ki + 8,
                                 kj:kj + 64]
                        lhsT = wof[:, cc, kk * 27:(kk + 1) * 27]
                        nc.tensor.matmul(
                            ps[:], lhsT=lhsT, rhs=rhs,
                            start=first, stop=(kk == KK - 1 and cc == 1))
                        first = False
                om_g = coord.tile([27, 512], F32, name="om_g", tag="om_g")
                nc.scalar.activation(om_g[:], ps[:],
                                     AF.Identity, bias=boff_t, scale=1.0)

                # --- omT for the 4 chunks of this group ---
                omT = coord.tile([128, 4, 27], F32, name="omT", tag="omT")
                for c in range(4):
                    pst = ps_misc.tile([128, 27], F32, name="pst",
                                       tag="psmisc")
                    nc.tensor.transpose(pst[:], om_g[:, c * 128:(c + 1) * 128],
                                        ident[:27, :27])
                    nc.vector.tensor_copy(omT[:, c, :], pst[:])

                # --- coords / weights / indices ([128, 36] tiles) ---
                _ntc = [0]

                def nt(shape=(128, FD), dt=F32):
                    _ntc[0] += 1
                    return coord.tile(list(shape), dt, name=f"ct{_ntc[0]}",
                                      tag=f"ct{_ntc[0]}")

                bsl = slice(g4 * FD, (g4 + 1) * FD)
                py = nt()
                px = nt()
                nc.vector.tensor_tensor(py[:], omT[:, :, 0:9],
                                        basey_t[:, bsl], op=ALU.add)
                nc.vector.tensor_tensor(px[:], omT[:, :, 9:18],
                                        basex_t[:, bsl], op=ALU.add)
                msk = nt()
                nc.scalar.activation(msk[:], omT[:, :, 18:27], AF.Sigmoid)
                # floor: HW f32->i32 convert rounds-to-nearest, so convert
                # (py - 0.5): round(py - 0.5) == floor(py) (coords > 0;
                # py - 0.5 is exact in fp32 at this magnitude)
                y0i = nt((128, FD), I32)
                x0i = nt((128, FD), I32)
                nc.vector.tensor_scalar(y0i[:], py[:], floor_bias, None,
                                        op0=ALU.add)
                nc.vector.tensor_scalar(x0i[:], px[:], floor_bias, None,
                                        op0=ALU.add)
                y0 = nt(); x0 = nt()
                nc.vector.tensor_copy(y0[:], y0i[:])
                nc.vector.tensor_copy(x0[:], x0i[:])
                ly = nt(); lx = nt()
                nc.vector.tensor_tensor(ly[:], py[:], y0[:], op=ALU.subtract)
                nc.vector.tensor_tensor(lx[:], px[:], x0[:], op=ALU.subtract)
                # weights: wtop = m*(1-ly), wbot = m*ly; w00 = wtop*(1-lx)...
                wbot = nt(); wtop = nt()
                nc.vector.tensor_tensor(wbot[:], ly[:], msk[:], op=ALU.mult)
                nc.vector.tensor_tensor(wtop[:], msk[:], wbot[:],
                                        op=ALU.subtract)
                t0 = nt(); t1 = nt()
                nc.vector.tensor_tensor(t0[:], wtop[:], lx[:], op=ALU.mult)
                nc.vector.tensor_tensor(t1[:], wbot[:], lx[:], op=ALU.mult)
                tw = {}
                for nm in CNAMES:
                    tw[nm] = nt((128, FD), F32)
                nc.vector.tensor_copy(tw["01"][:], t0[:])
                nc.vector.tensor_copy(tw["11"][:], t1[:])
                nc.vector.tensor_tensor(tw["00"][:], wtop[:], t0[:],
                                        op=ALU.subtract)
                nc.vector.tensor_tensor(tw["10"][:], wbot[:], t1[:],
                                        op=ALU.subtract)
                # f16 corner weights, k-major [128, KK, 4]
                wv16 = {}
                for nm in CNAMES:
                    wt = wvp.tile([128, KK, 4], F16, name=f"wv16{nm}",
                                  tag=f"wv16{nm}")
                    nc.vector.tensor_copy(
                        wt[:], tw[nm][:].rearrange("p (c k) -> p k c", k=KK))
                    wv16[nm] = wt
                # gather indices: idx = y0*Wp + x0 (+PAD offsets in base)
                idxf = nt()
                nc.vector.tensor_scalar(idxf[:], y0[:], float(Wp), None,
                                        op0=ALU.mult)
                nc.vector.tensor_tensor(idxf[:], idxf[:], x0[:], op=ALU.add)

                # 16-wrap the indices: two-stage PE transpose
                # idx16g[q, kk*32 + c*8 + r] (int16, stripe-replicated)
                idx16g = idxp.tile([128, KK * 32], I16, name="idx16g",
                                   tag="idx16g")
                idxv = idxf[:].rearrange("p (c k) -> p k c", k=KK)
                for kk in range(KK):
                    psa = ps_T.tile([4, 128], F32, name="psT1", tag="psT")
                    nc.tensor.transpose(psa[:], idxv[:, kk, :], ident[:])
                    a_sb = coord.tile([4, 128], F32, name="aT1", tag="aT1")
                    nc.vector.tensor_copy(a_sb[:], psa[:])
                    pst2 = ps_T.tile([16, 32], F32, name="psT2", tag="psT")
                    for r in range(8):
                        nc.tensor.transpose(pst2[:, r * 4:(r + 1) * 4],
                                            a_sb[:, r * 16:(r + 1) * 16],
                                            ident[:4, :4])
                    base = kk * 32
                    nc.vector.tensor_copy(
                        idx16g[0:16, base:base + 32]
                        .rearrange("q (c r) -> q c r", r=8),
                        pst2[:].rearrange("q (r c) -> q c r", r=8))
                # replicate this group's indices to all 8 gpsimd stripes
                for st in (16, 32, 64):
                    nc.sync.dma_start(out=idx16g[st:2 * st, :],
                                      in_=idx16g[0:st, :])
                return wv16, idx16g

            def compute_group(g4, wv16, idx16g):
                # --- gathers + scaled transposes + main conv ---
                psy = [ps_y.tile([128, 512], F32, name=f"psy{oc_}",
                                 tag=f"psy{oc_}") for oc_ in range(2)]
                for kk in range(KK):
                    s_sb = ssb.tile([128, 2, 512], MAIN_DT)
                    # gather 512 2x2-patch rows (4C f16 = 2KB each) in one
                    # dma_gather; output layout matches chunk/pixel-major
                    gt = gat.tile([128, 4, 4 * C], TABLE_DT)
                    nc.gpsimd.dma_gather(
                        out_ap=gt[:],
                        in_ap=table[:],
                        idxs_ap=idx16g[:, kk * 32:(kk + 1) * 32],
                        num_idxs=512, num_idxs_reg=512, elem_size=4 * C,
                        queue_num=(g4 * KK + kk) % 4)
                    # diag weight matrices (4 chunks per op, broadcast APs)
                    # + scaled transposes
                    ps_cc = [ps_s.tile([128, 512], F32, name=f"sps{cc_}",
                                       tag=f"sps{cc_}") for cc_ in range(2)]
                    dg4 = {}
                    for j, nm in enumerate(CNAMES):
                        d4 = diagp.tile([128, 4, 128], DIAG_DT,
                                        tag=f"diag{nm}")
                        nc.vector.tensor_tensor(
                            d4[:],
                            identd[:].rearrange("p (a f) -> p a f", a=1)
                            .to_broadcast([128, 4, 128]),
                            wv16[nm][:, kk, :]
                            .rearrange("p (c o) -> p c o", o=1)
                            .to_broadcast([128, 4, 128]),
                            op=ALU.mult)
                        dg4[(j // 2, j % 2)] = d4
                    for c4 in range(4):
                        for tb in range(2):
                            for xh in range(2):
                                for cc in range(2):
                                    base = (tb * 2 + xh) * 256 + cc * 128
                                    nc.tensor.matmul(
                                        ps_cc[cc][:, c4 * 128:(c4 + 1) * 128],
                                        lhsT=gt[:, c4, base:base + 128],
                                        rhs=dg4[(tb, xh)][:, c4, :],
                                        start=(c4 == 0 and tb == 0 and xh == 0),
                                        stop=(c4 == 3 and tb == 1 and xh == 1),
                                    )
                    for cc in range(2):
                        nc.scalar.activation(s_sb[:, cc, :], ps_cc[cc][:],
                                             AF.Copy)
                    # main conv contribution of this tap (PSUM-accumulated)
                    for oc in range(2):
                        for cc in range(2):
                            nc.tensor.matmul(
                                psy[oc][:],
                                lhsT=wmat_t[:, cc, kk * O + oc * 128:
                                            kk * O + (oc + 1) * 128],
                                rhs=s_sb[:, cc, :],
                                start=(kk == 0 and cc == 0),
                                stop=(kk == KK - 1 and cc == 1))

                y_sb = ysb.tile([128, 2, 512], F32)
                for oc in range(2):
                    nc.scalar.activation(y_sb[:, oc, :], psy[oc][:], AF.Relu,
                                         bias=shift_t[:, oc:oc + 1], scale=1.0)
                    nc.sync.dma_start(
                        out=yout[oc][:, g4 * 512:(g4 + 1) * 512],
                        in_=y_sb[:, oc, :])

            # software pipeline: prep group g4+1 before computing g4
            st = {0: prep_group(0)}
            for g4 in range(G4):
                if g4 + 1 < G4:
                    st[g4 + 1] = prep_group(g4 + 1)
                wv16_g, idx16g_g = st.pop(g4)
                compute_group(g4, wv16_g, idx16g_g)
    nc.compile()
    return nc


@functools.lru_cache(maxsize=1)
def _cached_nc():
    return build_nc()


def prep_core_inputs(inputs):
    """Host-side prep: per-core input maps (numpy only)."""
    x = np.asarray(inputs["x"], np.float32)
    w_off = np.asarray(inputs["w_off"], np.float32)
    b_off = np.asarray(inputs["b_off"], np.float32)
    w = np.asarray(inputs["w"], np.float32)
    b = np.asarray(inputs["b"], np.float32)
    gamma = np.asarray(inputs["gamma"], np.float32)
    beta = np.asarray(inputs["beta"], np.float32)
    rm = np.asarray(inputs["running_mean"], np.float32)
    rv = np.asarray(inputs["running_var"], np.float32)

    tdt = _np_dt(TABLE_DT)
    mdt = _np_dt(MAIN_DT)

    tables = []
    for bb in range(B):
        # flat padded pixel-major image with slack rows
        flat = np.zeros((VROWS + Wp + 2, C), np.float32)
        img = np.zeros((Hp, Wp, C), np.float32)
        img[PAD:PAD + H, PAD:PAD + W, :] = x[bb].transpose(1, 2, 0)
        flat[:Hp * Wp] = img.reshape(Hp * Wp, C)
        # patch table: row r = [r, r+1, r+Wp, r+Wp+1]
        t = np.empty((VROWS, 4 * C), tdt)
        t[:, 0 * C:1 * C] = flat[0:VROWS]
        t[:, 1 * C:2 * C] = flat[1:VROWS + 1]
        t[:, 2 * C:3 * C] = flat[Wp:VROWS + Wp]
        t[:, 3 * C:4 * C] = flat[Wp + 1:VROWS + Wp + 1]
        tables.append(np.ascontiguousarray(t))

    # offset conv weights: wofft[cc, p, kk, :] = w_off[:, cc*128+p, ki, kj]
    wofft = np.ascontiguousarray(
        w_off.reshape(27, 2, 128, 3, 3).transpose(1, 2, 3, 4, 0)
        .reshape(2, 128, 3 * 3 * 27)).astype(np.float32)

    # main conv weights, BN-folded: wmat[cc, p, kk, o] = w[o, cc*128+p, kk]*inv[o]
    inv = gamma / np.sqrt(rv + BN_EPS)
    shift = b * inv + beta - rm * inv
    wk = (w.reshape(O, C, KK) * inv[:, None, None]).astype(np.float32)
    wmat = np.ascontiguousarray(
        wk.reshape(O, 2, 128, KK).transpose(1, 2, 3, 0)).astype(mdt)

    shiftp = np.ascontiguousarray(shift.reshape(2, 128).T).astype(np.float32)
    boffp = np.zeros((128, 1), np.float32)
    boffp[:27, 0] = b_off

    in_maps = []
    for core in range(NCORES):
        bb = core // 2
        h0 = ROWS * (core % 2)
        # base grids [128, NCH*KK]: pixel n = c*128+p; h = h0 + n//64; w = n%64
        p = np.arange(128)
        cgrid = np.arange(NCH)
        kk = np.arange(KK)
        hh = (h0 + 2 * cgrid[None, :, None] + p[:, None, None] // 64)
        wwc = (p[:, None, None] % 64) * np.ones((1, NCH, 1))
        ki = (kk // 3)[None, None, :]
        kj = (kk % 3)[None, None, :]
        by = (hh - 1.0 + ki + PAD).astype(np.float32).reshape(128, NCH * KK)
        bx = (wwc - 1.0 + kj + PAD).astype(np.float32).reshape(128, NCH * KK)

        # xpad window rows h0-1 .. h0+32, zero-padded, 66 cols
        xp = np.zeros((2, 128, 34, 66), np.float32)
        r0 = max(0, h0 - 1); r1 = min(H, h0 + 33)
        xp[:, :, (r0 - (h0 - 1)):(r1 - (h0 - 1)), 1:W + 1] = (
            x[bb].reshape(2, 128, H, W)[:, :, r0:r1, :])

        # f16 blob [xpad 2*2244 | wofft 2*243]; f32 blob [by|bx|shift|boff]
        xw16 = np.concatenate([
            xp.reshape(2, 128, 34 * 66).transpose(1, 0, 2).reshape(128, -1),
            wofft.transpose(1, 0, 2).reshape(128, -1),
        ], axis=1).astype(np.float16)
        cblob = np.concatenate([by, bx, shiftp, boffp],
                               axis=1).astype(np.float32)

        in_maps.append(dict(
            table=tables[bb],
            xw16=np.ascontiguousarray(xw16),
            cblob=np.ascontiguousarray(cblob),
            wmat=wmat,
        ))
    return in_maps


def assemble_output(results):
    y = np.zeros((B, O, H, W), np.float32)
    for core in range(NCORES):
        bb = core // 2
        h0 = ROWS * (core % 2)
        yo = results[core]["yout"]  # [2, 128, N]
        y[bb, :, h0:h0 + ROWS, :] = yo.reshape(O, ROWS, W)
    return y


def kernel(**inputs):
    from concourse.bass_utils import run_bass_kernel_spmd
    nc = _cached_nc()
    in_maps = prep_core_inputs(inputs)
    res = run_bass_kernel_spmd(nc, in_maps, core_ids=list(range(NCORES)))
    return assemble_output(res.results)

